# revision 16
# baseline (speedup 1.0000x reference)
"""Trainium2 Bass kernel for nn_CenterLoss (retrieval_knn).

reference semantics (per batch b):
    dist[n, m] = ||pred[b, n] - gt[b, m]||^2           (N=4096, M=512)
    dist1[n] = min_m dist ; dist2[m] = min_n dist
    loss = sum(dist1*obj)/(sum(obj)+1e-6) + sum(dist2*mask)/(sum(mask)+1e-6)

Strategy: data-parallel over batch (16 batches -> 8 cores, 2 each). On each
core, per batch, the PE builds the NEGATED distance matrix T = -dist via a
K=5 augmented matmul:
    T[i, j] = sum_k pa[k, i] * ga[k, j]
    pa rows (pred side, negated): [-x, -y, -z, -|p|^2, -1]
    ga rows (gt side):            [-2gx, -2gy, -2gz, 1, |g|^2]
Negation lets every min become a max (the PE-transpose + free-dim reduce path
only needs max). Per 128-pred tile: DVE row-max of the PSUM tile gives
-dist1, and an elementwise max-accumulator (macc) over tiles collects the
per-gt column information; macc is then PE-transposed so the column max
(-dist2) is again a free-dim reduce. Masked sums reduce on-chip to 4 scalars
per batch; the final cross-partition sum is a 1-column matmul with ones.
Host combines the 8 cores' partial sums into the scalar loss.
"""

import numpy as np

B, N, M = 16, 4096, 512
N_CORES = 8
B_LOC = B // N_CORES        # batches per core
NT = N // 128               # pred tiles per batch
GT = M // 128               # gt blocks per batch

_PROGRAM_CACHE = {}


def _install_walrus_ctrl_wait_workaround():
    """The installed walrus rejects multi-wait CTRL (Drain) instructions
    ("Too many sync wait commands"). Split the TileContext end-of-kernel
    drain's sem waits onto individual NOPs (one wait each) on the same
    serial sync engine — semantically equivalent."""
    import concourse.tile as tile
    import concourse.mybir as mybir
    from concourse.vector_clock import ScopedClock

    if getattr(tile.TileContext, "_ctrl_wait_workaround", False):
        return

    def _drain_and_barrier(self, tick_clock, wait_clock):
        nc = self.nc
        drain_inst = nc.sync.drain()
        wait_clock.add_sem_waits(
            drain_inst.ins, ScopedClock({None: tick_clock.global_clock})
        )
        si = drain_inst.ins.sync_info
        if si is not None and si.on_wait and len(si.on_wait) > 1:
            waits = list(si.on_wait)
            si.on_wait.clear()
            for w in waits:
                nop_inst = nc.sync.nop()
                nop_inst.ins.sync_info = mybir.SyncInfo(on_wait=[w], on_update=[])

        nc.all_engine_barrier()
        assert self.sems is not None
        popped = nc._tile_sem_poison_stack.pop()
        assert popped is self._sem_poison
        nc.clear_and_free_semaphores(list(self.sems.allocated().values()))

    tile.TileContext._drain_and_barrier = _drain_and_barrier
    tile.TileContext._ctrl_wait_workaround = True


def _split_multi_waits_json(bir_bytes):
    """The installed walrus accepts at most one sem-wait per instruction.
    Rewrite the serialized BIR: any instruction carrying N>1 waits keeps its
    last wait and gets N-1 single-wait NoOps inserted just before it on the
    same (in-order) engine queue."""
    import orjson

    bir = orjson.loads(bir_bytes)
    counter = [0]
    for fn in bir["functions"]:
        for blk in fn["blocks"]:
            new_insts = []
            for ins in blk["instructions"]:
                si = ins.get("sync_info")
                if si and len(si.get("on_wait") or []) > 1:
                    waits = si["on_wait"]
                    for w in waits[:-1]:
                        counter[0] += 1
                        new_insts.append({
                            "debug": ins.get("debug"),
                            "engine": ins["engine"],
                            "ins": [],
                            "name": f"I-waitsplit-{counter[0]}",
                            "opcode": "NoOp",
                            "outs": [],
                            "sync_info": {"on_update": [], "on_wait": [w]},
                        })
                    si["on_wait"] = [waits[-1]]
                new_insts.append(ins)
            blk["instructions"] = new_insts
    return orjson.dumps(bir)


def _build_program():
    _install_walrus_ctrl_wait_workaround()
    import concourse.bass as bass
    import concourse.tile as tile
    from concourse import mybir
    from concourse.masks import make_identity

    f32 = mybir.dt.float32
    bf16 = mybir.dt.bfloat16
    f16 = mybir.dt.float16
    X = mybir.AxisListType.X
    mx = mybir.AluOpType.max
    mul = mybir.AluOpType.mult
    add = mybir.AluOpType.add

    nc = bass.Bass()
    pa_d = nc.declare_dram_parameter("pa", [B_LOC, 20, N], bf16, isOutput=False)
    ga_d = nc.declare_dram_parameter("ga", [B_LOC, 20, M], bf16, isOutput=False)
    obj_d = nc.declare_dram_parameter("obj", [B_LOC, 128, NT], f32, isOutput=False)
    msk_d = nc.declare_dram_parameter("msk", [B_LOC, 128, GT], f32, isOutput=False)
    out_d = nc.declare_dram_parameter("out", [B_LOC * 4], f32, isOutput=True)

    with tile.TileContext(nc) as tc:
        with (
            tc.tile_pool(name="consts", bufs=1) as consts,
            tc.tile_pool(name="inputs", bufs=2) as inputs,
            tc.tile_pool(name="work", bufs=2) as work,
            tc.tile_pool(name="mm", bufs=3, space="PSUM") as mm_pool,
            tc.tile_pool(name="tp", bufs=1, space="PSUM") as tp_pool,
            tc.tile_pool(name="fin", bufs=1, space="PSUM") as fin_pool,
        ):
            ident = consts.tile([128, 128], f16)
            make_identity(nc, ident[:])
            ones = consts.tile([128, 1], f32)
            nc.vector.memset(ones[:], 1.0)
            pp = consts.tile([128, B_LOC * 4], f32)

            for b in range(B_LOC):
                pa_sb = inputs.tile([20, N], bf16, tag="pa")
                nc.sync.dma_start(out=pa_sb[:], in_=pa_d[b])
                ga_sb = inputs.tile([20, M], bf16, tag="ga")
                nc.sync.dma_start(out=ga_sb[:], in_=ga_d[b])
                obj_sb = inputs.tile([128, NT], f32, tag="obj")
                nc.sync.dma_start(out=obj_sb[:], in_=obj_d[b])
                msk_sb = inputs.tile([128, GT], f32, tag="msk")
                nc.sync.dma_start(out=msk_sb[:], in_=msk_d[b])

                macc = work.tile([128, M], f16, tag="macc")
                nc.vector.memset(macc[:], -60000.0)
                d1 = work.tile([128, NT], f32, tag="d1")
                d2 = work.tile([128, GT], f32, tag="d2")

                for g in range(NT // 8):
                    x8 = work.tile([128, 8, M], f16, tag="x8")
                    for i in range(4):
                        ps2 = mm_pool.tile([128, 2, M], f32, tag="mmt")
                        for j in range(2):
                            t = g * 8 + 2 * i + j
                            nc.tensor.matmul(
                                ps2[:, j, :],
                                pa_sb[:, t * 128 : (t + 1) * 128],
                                ga_sb[:],
                                start=True,
                                stop=True,
                            )
                        # ACT: PSUM fp32 -> SBUF fp16, two tiles per op
                        nc.scalar.copy(
                            out=x8[:, 2 * i : 2 * i + 2, :], in_=ps2[:]
                        )
                        # column accumulator for -dist2 (fp16 2x mode)
                        nc.vector.tensor_tensor(
                            out=macc[:], in0=macc[:], in1=x8[:, 2 * i, :], op=mx
                        )
                        nc.vector.tensor_tensor(
                            out=macc[:], in0=macc[:], in1=x8[:, 2 * i + 1, :], op=mx
                        )
                    # -dist1 for these 8x128 preds: vectorized max tree
                    t1 = work.tile([128, 8, 256], f16, tag="t1")
                    nc.vector.tensor_tensor(
                        out=t1[:], in0=x8[:, :, 0:256], in1=x8[:, :, 256:512], op=mx
                    )
                    t2 = work.tile([128, 8, 128], f16, tag="t2")
                    nc.vector.tensor_tensor(
                        out=t2[:], in0=t1[:, :, 0:128], in1=t1[:, :, 128:256], op=mx
                    )
                    t3 = work.tile([128, 8, 64], f16, tag="t3")
                    nc.vector.tensor_tensor(
                        out=t3[:], in0=t2[:, :, 0:64], in1=t2[:, :, 64:128], op=mx
                    )
                    t4 = work.tile([128, 8, 32], f16, tag="t4")
                    nc.vector.tensor_tensor(
                        out=t4[:], in0=t3[:, :, 0:32], in1=t3[:, :, 32:64], op=mx
                    )
                    t5 = work.tile([128, 8, 16], f16, tag="t5")
                    nc.vector.tensor_tensor(
                        out=t5[:], in0=t4[:, :, 0:16], in1=t4[:, :, 16:32], op=mx
                    )
                    nc.vector.tensor_reduce(
                        out=d1[:, g * 8 : (g + 1) * 8], in_=t5[:], axis=X, op=mx
                    )

                # -dist2: transpose macc 128-blocks, then free-dim max
                for k in range(GT):
                    tp = tp_pool.tile([128, 128], f16, tag="tpt")
                    nc.tensor.transpose(
                        tp[:], macc[:, k * 128 : (k + 1) * 128], ident[:]
                    )
                    nc.vector.tensor_reduce(
                        out=d2[:, k : k + 1], in_=tp[:], axis=X, op=mx
                    )

                # per-partition partials: [-S1, sum(obj), -S2, sum(mask)]
                j1 = work.tile([128, NT], f32, tag="j1")
                nc.vector.tensor_tensor(out=j1[:], in0=d1[:], in1=obj_sb[:], op=mul)
                nc.vector.tensor_reduce(
                    out=pp[:, 4 * b + 0 : 4 * b + 1], in_=j1[:], axis=X, op=add
                )
                nc.vector.tensor_reduce(
                    out=pp[:, 4 * b + 1 : 4 * b + 2], in_=obj_sb[:], axis=X, op=add
                )
                j2 = work.tile([128, GT], f32, tag="j2")
                nc.vector.tensor_tensor(out=j2[:], in0=d2[:], in1=msk_sb[:], op=mul)
                nc.vector.tensor_reduce(
                    out=pp[:, 4 * b + 2 : 4 * b + 3], in_=j2[:], axis=X, op=add
                )
                nc.vector.tensor_reduce(
                    out=pp[:, 4 * b + 3 : 4 * b + 4], in_=msk_sb[:], axis=X, op=add
                )

            # cross-partition sum of all partials in one 1-column matmul
            po = fin_pool.tile([B_LOC * 4, 1], f32)
            nc.tensor.matmul(po[:], pp[:], ones[:], start=True, stop=True)
            po_sb = consts.tile([B_LOC * 4, 1], f32)
            nc.vector.tensor_copy(out=po_sb[:], in_=po[:])
            nc.sync.dma_start(out=out_d[:], in_=po_sb[:, 0])

    _orig_to_json_bytes = nc.to_json_bytes
    nc.to_json_bytes = lambda: _split_multi_waits_json(_orig_to_json_bytes())
    return nc


def _get_program():
    if "nc" not in _PROGRAM_CACHE:
        _PROGRAM_CACHE["nc"] = _build_program()
    return _PROGRAM_CACHE["nc"]


def _hi_lo_split(x, bf16):
    hi = x.astype(bf16)
    lo = (x - hi.astype(np.float32)).astype(bf16)
    return hi, lo


def _prep_core_inputs(pred, gt, obj, mask):
    """pred (B_LOC,N,3) gt (B_LOC,M,3) obj (B_LOC,N) int32 mask (B_LOC,M).

    The matmul runs in bf16 with a hi/lo split (K=20): the four hi/lo row
    groups reproduce the fp32 dot products to ~2^-18 at bf16 PE speed."""
    import ml_dtypes
    bf16 = ml_dtypes.bfloat16

    pred = np.asarray(pred, np.float32)
    gt = np.asarray(gt, np.float32)
    pa = np.empty((B_LOC, 5, N), np.float32)
    pa[:, 0:3] = -pred.transpose(0, 2, 1)
    pa[:, 3] = -np.square(pred).sum(-1)
    pa[:, 4] = -1.0
    ga = np.empty((B_LOC, 5, M), np.float32)
    ga[:, 0:3] = -2.0 * gt.transpose(0, 2, 1)
    ga[:, 3] = 1.0
    ga[:, 4] = np.square(gt).sum(-1)

    pa_hi, pa_lo = _hi_lo_split(pa, bf16)
    ga_hi, ga_lo = _hi_lo_split(ga, bf16)
    pa20 = np.concatenate([pa_hi, pa_hi, pa_lo, pa_lo], axis=1)
    ga20 = np.concatenate([ga_hi, ga_lo, ga_hi, ga_lo], axis=1)

    ob = np.ascontiguousarray(
        np.asarray(obj, np.float32).reshape(B_LOC, NT, 128).transpose(0, 2, 1)
    )
    mk = np.ascontiguousarray(
        np.asarray(mask, np.float32).reshape(B_LOC, GT, 128).transpose(0, 2, 1)
    )
    return {"pa": pa20, "ga": ga20, "obj": ob, "msk": mk}


def run(pred_center, center_label, box_label_mask, objectness_label, trace=False):
    """Run the sharded kernel; returns (loss_scalar, BassKernelResults)."""
    from concourse.bass_utils import run_bass_kernel_spmd

    nc = _get_program()
    in_maps = []
    for c in range(N_CORES):
        bs = slice(B_LOC * c, B_LOC * (c + 1))
        in_maps.append(
            _prep_core_inputs(
                pred_center[bs], center_label[bs],
                objectness_label[bs], box_label_mask[bs],
            )
        )
    res = run_bass_kernel_spmd(nc, in_maps, list(range(N_CORES)), trace=trace)
    q = np.stack(
        [res.results[c]["out"].reshape(B_LOC, 4) for c in range(N_CORES)]
    ).astype(np.float64)
    s1 = -q[..., 0].sum()
    sum_obj = q[..., 1].sum()
    s2 = -q[..., 2].sum()
    sum_mask = q[..., 3].sum()
    loss = s1 / (sum_obj + 1e-6) + s2 / (sum_mask + 1e-6)
    return np.float32(loss), res


def kernel(pred_center, center_label, box_label_mask, objectness_label):
    loss, _ = run(pred_center, center_label, box_label_mask, objectness_label)
    return np.array(loss, dtype=np.float32)


# revision 23
# speedup vs baseline: 1.0029x; 1.0029x over previous
"""Trainium2 Bass kernel for nn_CenterLoss (retrieval_knn).

reference semantics (per batch b):
    dist[n, m] = ||pred[b, n] - gt[b, m]||^2           (N=4096, M=512)
    dist1[n] = min_m dist ; dist2[m] = min_n dist
    loss = sum(dist1*obj)/(sum(obj)+1e-6) + sum(dist2*mask)/(sum(mask)+1e-6)

Strategy: data-parallel over batch (16 batches -> 8 cores, 2 each). On each
core, per batch, the PE builds the NEGATED distance matrix T = -dist via a
K=5 augmented matmul:
    T[i, j] = sum_k pa[k, i] * ga[k, j]
    pa rows (pred side, negated): [-x, -y, -z, -|p|^2, -1]
    ga rows (gt side):            [-2gx, -2gy, -2gz, 1, |g|^2]
Negation lets every min become a max (the PE-transpose + free-dim reduce path
only needs max). Per 128-pred tile: DVE row-max of the PSUM tile gives
-dist1, and an elementwise max-accumulator (macc) over tiles collects the
per-gt column information; macc is then PE-transposed so the column max
(-dist2) is again a free-dim reduce. Masked sums reduce on-chip to 4 scalars
per batch; the final cross-partition sum is a 1-column matmul with ones.
Host combines the 8 cores' partial sums into the scalar loss.
"""

import numpy as np

B, N, M = 16, 4096, 512
N_CORES = 8
B_LOC = B // N_CORES        # batches per core
NT = N // 128               # pred tiles per batch
GT = M // 128               # gt blocks per batch

_PROGRAM_CACHE = {}


def _install_walrus_ctrl_wait_workaround():
    """The installed walrus rejects multi-wait CTRL (Drain) instructions
    ("Too many sync wait commands"). Split the TileContext end-of-kernel
    drain's sem waits onto individual NOPs (one wait each) on the same
    serial sync engine — semantically equivalent."""
    import concourse.tile as tile
    import concourse.mybir as mybir
    from concourse.vector_clock import ScopedClock

    if getattr(tile.TileContext, "_ctrl_wait_workaround", False):
        return

    def _drain_and_barrier(self, tick_clock, wait_clock):
        nc = self.nc
        drain_inst = nc.sync.drain()
        wait_clock.add_sem_waits(
            drain_inst.ins, ScopedClock({None: tick_clock.global_clock})
        )
        si = drain_inst.ins.sync_info
        if si is not None and si.on_wait and len(si.on_wait) > 1:
            waits = list(si.on_wait)
            si.on_wait.clear()
            engines = [nc.sync, nc.vector, nc.scalar, nc.tensor, nc.gpsimd]
            for idx, w in enumerate(waits):
                nop_inst = engines[idx % len(engines)].nop()
                nop_inst.ins.sync_info = mybir.SyncInfo(on_wait=[w], on_update=[])

        nc.all_engine_barrier()
        assert self.sems is not None
        popped = nc._tile_sem_poison_stack.pop()
        assert popped is self._sem_poison
        nc.clear_and_free_semaphores(list(self.sems.allocated().values()))

    tile.TileContext._drain_and_barrier = _drain_and_barrier
    tile.TileContext._ctrl_wait_workaround = True


def _split_multi_waits_json(bir_bytes):
    """The installed walrus accepts at most one sem-wait per instruction.
    Rewrite the serialized BIR: any instruction carrying N>1 waits keeps its
    last wait and gets N-1 single-wait NoOps inserted just before it on the
    same (in-order) engine queue."""
    import orjson

    bir = orjson.loads(bir_bytes)
    counter = [0]
    for fn in bir["functions"]:
        for blk in fn["blocks"]:
            new_insts = []
            for ins in blk["instructions"]:
                si = ins.get("sync_info")
                if si and len(si.get("on_wait") or []) > 1:
                    waits = si["on_wait"]
                    for w in waits[:-1]:
                        counter[0] += 1
                        new_insts.append({
                            "debug": ins.get("debug"),
                            "engine": ins["engine"],
                            "ins": [],
                            "name": f"I-waitsplit-{counter[0]}",
                            "opcode": "NoOp",
                            "outs": [],
                            "sync_info": {"on_update": [], "on_wait": [w]},
                        })
                    si["on_wait"] = [waits[-1]]
                new_insts.append(ins)
            blk["instructions"] = new_insts
    return orjson.dumps(bir)


def _build_program():
    _install_walrus_ctrl_wait_workaround()
    import concourse.bass as bass
    import concourse.tile as tile
    from concourse import mybir
    from concourse.masks import make_identity

    f32 = mybir.dt.float32
    bf16 = mybir.dt.bfloat16
    f16 = mybir.dt.float16
    X = mybir.AxisListType.X
    mx = mybir.AluOpType.max
    mul = mybir.AluOpType.mult
    add = mybir.AluOpType.add

    nc = bass.Bass()
    pa_d = nc.declare_dram_parameter("pa", [B_LOC, 20, N], bf16, isOutput=False)
    ga_d = nc.declare_dram_parameter("ga", [B_LOC, 20, M], bf16, isOutput=False)
    obj_d = nc.declare_dram_parameter("obj", [B_LOC, 128, NT], f32, isOutput=False)
    msk_d = nc.declare_dram_parameter("msk", [B_LOC, 128, GT], f32, isOutput=False)
    out_d = nc.declare_dram_parameter("out", [B_LOC * 4], f32, isOutput=True)

    with tile.TileContext(nc) as tc:
        with (
            tc.tile_pool(name="consts", bufs=1) as consts,
            tc.tile_pool(name="inputs", bufs=2) as inputs,
            tc.tile_pool(name="work", bufs=2) as work,
            tc.tile_pool(name="mm", bufs=3, space="PSUM") as mm_pool,
            tc.tile_pool(name="tp", bufs=1, space="PSUM") as tp_pool,
            tc.tile_pool(name="fin", bufs=1, space="PSUM") as fin_pool,
        ):
            ident = consts.tile([128, 128], f16)
            make_identity(nc, ident[:])
            ones = consts.tile([128, 1], f32)
            nc.vector.memset(ones[:], 1.0)
            pp = consts.tile([128, B_LOC * 4], f32)

            for b in range(B_LOC):
                pa_sb = inputs.tile([20, N], bf16, tag="pa")
                nc.sync.dma_start(out=pa_sb[:], in_=pa_d[b])
                ga_sb = inputs.tile([20, M], bf16, tag="ga")
                nc.sync.dma_start(out=ga_sb[:], in_=ga_d[b])
                obj_sb = inputs.tile([128, NT], f32, tag="obj")
                nc.sync.dma_start(out=obj_sb[:], in_=obj_d[b])
                msk_sb = inputs.tile([128, GT], f32, tag="msk")
                nc.sync.dma_start(out=msk_sb[:], in_=msk_d[b])

                macc = work.tile([128, M], f16, tag="macc")
                nc.vector.memset(macc[:], -60000.0)
                d1 = work.tile([128, NT], f32, tag="d1")
                d2 = work.tile([128, GT], f32, tag="d2")

                for g in range(NT // 8):
                    x8 = work.tile([128, 8, M], f16, tag="x8")
                    for i in range(4):
                        ps2 = mm_pool.tile([128, 2, M], f32, tag="mmt")
                        for j in range(2):
                            t = g * 8 + 2 * i + j
                            nc.tensor.matmul(
                                ps2[:, j, :],
                                pa_sb[:, t * 128 : (t + 1) * 128],
                                ga_sb[:],
                                start=True,
                                stop=True,
                            )
                        # ACT: PSUM fp32 -> SBUF fp16, two tiles per op
                        nc.scalar.copy(
                            out=x8[:, 2 * i : 2 * i + 2, :], in_=ps2[:]
                        )
                        # column accumulator for -dist2 (fp16 2x mode)
                        nc.vector.tensor_tensor(
                            out=macc[:], in0=macc[:], in1=x8[:, 2 * i, :], op=mx
                        )
                        nc.vector.tensor_tensor(
                            out=macc[:], in0=macc[:], in1=x8[:, 2 * i + 1, :], op=mx
                        )
                    # -dist1 for these 8x128 preds: vectorized max tree
                    t1 = work.tile([128, 8, 256], f16, tag="t1")
                    nc.vector.tensor_tensor(
                        out=t1[:], in0=x8[:, :, 0:256], in1=x8[:, :, 256:512], op=mx
                    )
                    t2 = work.tile([128, 8, 128], f16, tag="t2")
                    nc.vector.tensor_tensor(
                        out=t2[:], in0=t1[:, :, 0:128], in1=t1[:, :, 128:256], op=mx
                    )
                    t3 = work.tile([128, 8, 64], f16, tag="t3")
                    nc.vector.tensor_tensor(
                        out=t3[:], in0=t2[:, :, 0:64], in1=t2[:, :, 64:128], op=mx
                    )
                    t4 = work.tile([128, 8, 32], f16, tag="t4")
                    nc.vector.tensor_tensor(
                        out=t4[:], in0=t3[:, :, 0:32], in1=t3[:, :, 32:64], op=mx
                    )
                    t5 = work.tile([128, 8, 16], f16, tag="t5")
                    nc.vector.tensor_tensor(
                        out=t5[:], in0=t4[:, :, 0:16], in1=t4[:, :, 16:32], op=mx
                    )
                    nc.vector.tensor_reduce(
                        out=d1[:, g * 8 : (g + 1) * 8], in_=t5[:], axis=X, op=mx
                    )

                # -dist2: transpose macc 128-blocks, then free-dim max
                for k in range(GT):
                    tp = tp_pool.tile([128, 128], f16, tag="tpt")
                    nc.tensor.transpose(
                        tp[:], macc[:, k * 128 : (k + 1) * 128], ident[:]
                    )
                    nc.vector.tensor_reduce(
                        out=d2[:, k : k + 1], in_=tp[:], axis=X, op=mx
                    )

                # per-partition partials: [-S1, sum(obj), -S2, sum(mask)]
                j1 = work.tile([128, NT], f32, tag="j1")
                nc.vector.tensor_tensor(out=j1[:], in0=d1[:], in1=obj_sb[:], op=mul)
                nc.vector.tensor_reduce(
                    out=pp[:, 4 * b + 0 : 4 * b + 1], in_=j1[:], axis=X, op=add
                )
                nc.vector.tensor_reduce(
                    out=pp[:, 4 * b + 1 : 4 * b + 2], in_=obj_sb[:], axis=X, op=add
                )
                j2 = work.tile([128, GT], f32, tag="j2")
                nc.vector.tensor_tensor(out=j2[:], in0=d2[:], in1=msk_sb[:], op=mul)
                nc.vector.tensor_reduce(
                    out=pp[:, 4 * b + 2 : 4 * b + 3], in_=j2[:], axis=X, op=add
                )
                nc.vector.tensor_reduce(
                    out=pp[:, 4 * b + 3 : 4 * b + 4], in_=msk_sb[:], axis=X, op=add
                )

            # cross-partition sum of all partials in one 1-column matmul
            po = fin_pool.tile([B_LOC * 4, 1], f32)
            nc.tensor.matmul(po[:], pp[:], ones[:], start=True, stop=True)
            po_sb = consts.tile([B_LOC * 4, 1], f32)
            nc.vector.tensor_copy(out=po_sb[:], in_=po[:])
            nc.sync.dma_start(out=out_d[:], in_=po_sb[:, 0])

    _orig_to_json_bytes = nc.to_json_bytes
    nc.to_json_bytes = lambda: _split_multi_waits_json(_orig_to_json_bytes())
    return nc


def _get_program():
    if "nc" not in _PROGRAM_CACHE:
        _PROGRAM_CACHE["nc"] = _build_program()
    return _PROGRAM_CACHE["nc"]


def _hi_lo_split(x, bf16):
    hi = x.astype(bf16)
    lo = (x - hi.astype(np.float32)).astype(bf16)
    return hi, lo


def _prep_core_inputs(pred, gt, obj, mask):
    """pred (B_LOC,N,3) gt (B_LOC,M,3) obj (B_LOC,N) int32 mask (B_LOC,M).

    The matmul runs in bf16 with a hi/lo split (K=20): the four hi/lo row
    groups reproduce the fp32 dot products to ~2^-18 at bf16 PE speed."""
    import ml_dtypes
    bf16 = ml_dtypes.bfloat16

    pred = np.asarray(pred, np.float32)
    gt = np.asarray(gt, np.float32)
    pa = np.empty((B_LOC, 5, N), np.float32)
    pa[:, 0:3] = -pred.transpose(0, 2, 1)
    pa[:, 3] = -np.square(pred).sum(-1)
    pa[:, 4] = -1.0
    ga = np.empty((B_LOC, 5, M), np.float32)
    ga[:, 0:3] = -2.0 * gt.transpose(0, 2, 1)
    ga[:, 3] = 1.0
    ga[:, 4] = np.square(gt).sum(-1)

    pa_hi, pa_lo = _hi_lo_split(pa, bf16)
    ga_hi, ga_lo = _hi_lo_split(ga, bf16)
    pa20 = np.concatenate([pa_hi, pa_hi, pa_lo, pa_lo], axis=1)
    ga20 = np.concatenate([ga_hi, ga_lo, ga_hi, ga_lo], axis=1)

    ob = np.ascontiguousarray(
        np.asarray(obj, np.float32).reshape(B_LOC, NT, 128).transpose(0, 2, 1)
    )
    mk = np.ascontiguousarray(
        np.asarray(mask, np.float32).reshape(B_LOC, GT, 128).transpose(0, 2, 1)
    )
    return {"pa": pa20, "ga": ga20, "obj": ob, "msk": mk}


def run(pred_center, center_label, box_label_mask, objectness_label, trace=False):
    """Run the sharded kernel; returns (loss_scalar, BassKernelResults)."""
    from concourse.bass_utils import run_bass_kernel_spmd

    nc = _get_program()
    in_maps = []
    for c in range(N_CORES):
        bs = slice(B_LOC * c, B_LOC * (c + 1))
        in_maps.append(
            _prep_core_inputs(
                pred_center[bs], center_label[bs],
                objectness_label[bs], box_label_mask[bs],
            )
        )
    res = run_bass_kernel_spmd(nc, in_maps, list(range(N_CORES)), trace=trace)
    q = np.stack(
        [res.results[c]["out"].reshape(B_LOC, 4) for c in range(N_CORES)]
    ).astype(np.float64)
    s1 = -q[..., 0].sum()
    sum_obj = q[..., 1].sum()
    s2 = -q[..., 2].sum()
    sum_mask = q[..., 3].sum()
    loss = s1 / (sum_obj + 1e-6) + s2 / (sum_mask + 1e-6)
    return np.float32(loss), res


def kernel(pred_center, center_label, box_label_mask, objectness_label):
    loss, _ = run(pred_center, center_label, box_label_mask, objectness_label)
    return np.array(loss, dtype=np.float32)
